# revision 13
# baseline (speedup 1.0000x reference)
"""Distance-aware multihead attention on 8 Trainium2 NeuronCores (v3).

Problem: B=4, S=1024, D=768, H=12, DK=64, NUM_EMB=10.
  q/k/v = linear projections of query/key/value
  idx[b,i,j] = clip(round(9 * |pos_i - pos_j| / MAXD), 0, 9)
  logits = (q.k^T + qe[b,h,i,idx[b,i,j]]) / 8   where qe = q @ emb_k^T
  out = softmax(logits) @ v

Key decompositions:
  - bias qe[...,idx] = sum_{e=1..9} (qe_e - qe_{e-1}) * (d2 >= T_e^2); the
    qe_0 term is constant along the softmax axis and cancels.
  - step masks (d2 >= T_e^2) are shared across all 12 heads of a q-tile.
  - bias applied on the TENSOR engine: 9 diag(dqe_e) @ mask_e matmuls
    accumulated into the same PSUM group as the QK matmul (a diag matmul is
    a per-partition row scaling).  No per-element vector work for the bias.

v3: X and W arrive from the host already transposed ([in_dim, token] /
[in_dim, out_dim]) and cast to fp16, so DMA loads land in the exact SBUF
layout the PE needs - no on-chip transposition of inputs at all.  All PE
operands fp16 except P/V (bf16 for exp range).  Head loop is software-
pipelined (PE group for head h+1 issues before the transpose/AV tail of
head h).

Sharding: core c handles batch c//2, query-half c%2 (512 queries, all heads).
"""
import numpy as np

import concourse.bass as bass
import concourse.tile as tile
from concourse import bacc, mybir
from concourse.bass_utils import run_bass_kernel_spmd

F32 = mybir.dt.float32
BF16 = mybir.dt.bfloat16
FP16 = mybir.dt.float16
ACT = mybir.ActivationFunctionType
ALU = mybir.AluOpType

B, S, D = 4, 1024, 768
H, DK = 12, 64
NUM_EMB = 10
MAX_DIST = 100000.0 * 2 ** 0.5
SQ = S // 2          # queries per core
NQT = SQ // 128      # q-tiles per core (4)
NKT = S // 128       # k token chunks (8)
NDT = D // 128       # dim tiles (6)
NCORES = 8

# squared thresholds: idx >= e  <=>  d2 >= ((e-0.5)*MAX_DIST/9)^2
THRESH2 = [float(((e - 0.5) * MAX_DIST / 9.0) ** 2) for e in range(1, NUM_EMB)]


def build_nc():
    nc = bacc.Bacc("TRN2", target_bir_lowering=False, debug=False)

    # host-pretransposed fp16 operands: [contraction_dim, free_dim]
    xqT_d = nc.dram_tensor("xqT", [D, SQ], FP16, kind="ExternalInput").ap()
    xkT_d = nc.dram_tensor("xkT", [D, S], FP16, kind="ExternalInput").ap()
    xvT_d = nc.dram_tensor("xvT", [D, S], FP16, kind="ExternalInput").ap()
    wqT_d = nc.dram_tensor("wqT", [D, D], FP16, kind="ExternalInput").ap()
    wkT_d = nc.dram_tensor("wkT", [D, D], FP16, kind="ExternalInput").ap()
    wvT_d = nc.dram_tensor("wvT", [D, D], FP16, kind="ExternalInput").ap()
    embT_d = nc.dram_tensor("embT", [DK, NUM_EMB], FP16, kind="ExternalInput").ap()
    pos = nc.dram_tensor("pos", [S, 2], F32, kind="ExternalInput").ap()
    posq = nc.dram_tensor("posq", [SQ, 2], F32, kind="ExternalInput").ap()
    bq = nc.dram_tensor("bq", [D], F32, kind="ExternalInput").ap()
    bk = nc.dram_tensor("bk", [D], F32, kind="ExternalInput").ap()
    bv = nc.dram_tensor("bv", [D], F32, kind="ExternalInput").ap()
    out = nc.dram_tensor("out", [SQ, D], F32, kind="ExternalOutput").ap()

    with tile.TileContext(nc) as tc:
        with tc.tile_pool(name="persist", bufs=1) as persist:
            from concourse.masks import make_identity
            ident16 = persist.tile([128, 128], FP16)
            identb = persist.tile([128, 128], BF16)
            make_identity(nc, ident16[:])
            make_identity(nc, identb[:])

            # small loads on the Pool SWDGE queue
            bq_col = persist.tile([128, NDT], F32)
            bk_col = persist.tile([128, NDT], F32)
            nc.gpsimd.dma_start(out=bq_col[:], in_=bass.AP(tensor=bq.tensor, offset=0, ap=[[1, 128], [128, NDT]]))
            nc.gpsimd.dma_start(out=bk_col[:], in_=bass.AP(tensor=bk.tensor, offset=0, ap=[[1, 128], [128, NDT]]))
            bv_b = persist.tile([128, D], F32)
            nc.gpsimd.dma_start(out=bv_b[:], in_=bass.AP(tensor=bv.tensor, offset=0, ap=[[0, 128], [1, D]]))
            # position x/y as single-partition rows [1, S]
            posx_row = persist.tile([1, S], F32)
            posy_row = persist.tile([1, S], F32)
            nc.gpsimd.dma_start(out=posx_row[:], in_=bass.AP(tensor=pos.tensor, offset=0, ap=[[2, 1], [2, S]]))
            nc.gpsimd.dma_start(out=posy_row[:], in_=bass.AP(tensor=pos.tensor, offset=1, ap=[[2, 1], [2, S]]))
            # query positions as per-partition scalars [128, NQT]
            xq_col = persist.tile([128, NQT], F32)
            yq_col = persist.tile([128, NQT], F32)
            nc.gpsimd.dma_start(out=xq_col[:], in_=bass.AP(tensor=posq.tensor, offset=0, ap=[[2, 128], [256, NQT]]))
            nc.gpsimd.dma_start(out=yq_col[:], in_=bass.AP(tensor=posq.tensor, offset=1, ap=[[2, 128], [256, NQT]]))
            # emb^T block-diagonal [128, 2*NUM_EMB] fp16 (2 heads per matmul)
            embT_blk = persist.tile([128, 2 * NUM_EMB], FP16)
            nc.vector.memset(embT_blk[:], 0.0)
            nc.gpsimd.dma_start(out=embT_blk[0:64, 0:NUM_EMB], in_=embT_d[:, :])
            nc.gpsimd.dma_start(out=embT_blk[64:128, NUM_EMB:2 * NUM_EMB], in_=embT_d[:, :])

            ones1 = persist.tile([1, 128], F32)
            nc.vector.memset(ones1[:], 1.0)

            # persistent attention operands
            kT = persist.tile([128, NDT, S], FP16)      # K^T[dim, token] + bk
            qT = persist.tile([128, NDT, SQ], FP16)     # Q^T[dim, token] + bq
            v_sb = persist.tile([128, NKT, D], BF16)    # V[token, dim] (no bias)
            xk_b = persist.tile([128, S], F32)          # pos-x broadcast rows
            yk_b = persist.tile([128, S], F32)
            dqe = persist.tile([128, NQT, H, NUM_EMB - 1], F32)

            # ---- load (already transposed), project, attention ----
            # PSUM budget (8 banks): pj 2 + qk 4 + ptp 1 + av 1
            with tc.tile_pool(name="tsp", bufs=2) as tsp, \
                 tc.tile_pool(name="pj_ps", bufs=2, space="PSUM") as pj_ps, \
                 tc.tile_pool(name="att", bufs=2) as att, \
                 tc.tile_pool(name="prep", bufs=2) as prep, \
                 tc.tile_pool(name="qk_ps", bufs=2, space="PSUM") as qk_ps, \
                 tc.tile_pool(name="pt_ps", bufs=1, space="PSUM") as pt_ps, \
                 tc.tile_pool(name="av_ps", bufs=1, space="PSUM") as av_ps:

                # broadcast pos rows across partitions via 1-partition matmul
                for dst, row in ((xk_b, posx_row), (yk_b, posy_row)):
                    for hf in range(2):
                        sl = slice(512 * hf, 512 * hf + 512)
                        bc = pj_ps.tile([128, 512], F32, tag="pj")
                        nc.tensor.matmul(bc[:], ones1[:], row[:, sl],
                                         start=True, stop=True)
                        nc.scalar.copy(dst[:, sl], bc[:])

                def load_T(src, dst, ncols, dma_eng):
                    for d in range(NDT):
                        dma_eng.dma_start(out=dst[:, d, 0:ncols],
                                          in_=src[128 * d:128 * (d + 1), :])

                # Q first: attention fronts need qT/dqe earliest
                xqT = tsp.tile([128, NDT, S], FP16, tag="xT")
                wqT = tsp.tile([128, NDT, D], FP16, tag="wT")
                load_T(xqT_d, xqT, SQ, nc.sync)
                load_T(wqT_d, wqT, D, nc.scalar)
                for m in range(NDT):
                    ps = pj_ps.tile([128, 512], F32, tag="pj")
                    for t in range(NDT):
                        nc.tensor.matmul(ps[:], wqT[:, t, 128 * m:128 * m + 128],
                                         xqT[:, t, 0:SQ],
                                         start=(t == 0), stop=(t == NDT - 1))
                    nc.scalar.activation(qT[:, m, :], ps[:], ACT.Identity,
                                         bias=bq_col[:, m:m + 1])

                # qe for all q-tiles: block-diag emb matmul, 2 heads per 128-dim block
                qe_psum = pj_ps.tile([128, 512], F32, tag="pj")
                for qt in range(NQT):
                    for m in range(NDT):
                        nc.tensor.matmul(qe_psum[:, 120 * qt + 20 * m:120 * qt + 20 * m + 20],
                                         qT[:, m, 128 * qt:128 * qt + 128],
                                         embT_blk[:],
                                         start=True, stop=True)
                qe_sb = persist.tile([128, NQT, H, NUM_EMB], F32)
                nc.scalar.copy(qe_sb[:], qe_psum[:, 0:NQT * H * NUM_EMB]
                               .rearrange("p (q h e) -> p q h e", e=NUM_EMB, h=H))
                nc.vector.tensor_tensor(out=dqe[:], in0=qe_sb[:, :, :, 1:],
                                        in1=qe_sb[:, :, :, :-1], op=ALU.subtract)

                # K
                xkT = tsp.tile([128, NDT, S], FP16, tag="xT")
                wkT = tsp.tile([128, NDT, D], FP16, tag="wT")
                load_T(xkT_d, xkT, S, nc.sync)
                load_T(wkT_d, wkT, D, nc.scalar)
                for m in range(NDT):
                    for hf in range(2):
                        ps = pj_ps.tile([128, 512], F32, tag="pj")
                        for t in range(NDT):
                            nc.tensor.matmul(ps[:], wkT[:, t, 128 * m:128 * m + 128],
                                             xkT[:, t, 512 * hf:512 * hf + 512],
                                             start=(t == 0), stop=(t == NDT - 1))
                        nc.scalar.activation(kT[:, m, 512 * hf:512 * hf + 512], ps[:],
                                             ACT.Identity, bias=bk_col[:, m:m + 1])

                # V: loads issued now; the 16 projection groups are emitted
                # interleaved into qt0's head loop (PE work hidden under attention)
                xvT = tsp.tile([128, NDT, S], FP16, tag="xT")
                wvT = tsp.tile([128, NDT, D], FP16, tag="wT")
                load_T(xvT_d, xvT, S, nc.sync)
                load_T(wvT_d, wvT, D, nc.scalar)

                def make_vgroup(m, hf):
                    def emit():
                        ps = pj_ps.tile([128, 512], F32, tag="pj")
                        for t in range(NDT):
                            nc.tensor.matmul(ps[:, 0:384], xvT[:, t, 128 * m:128 * m + 128],
                                             wvT[:, t, 384 * hf:384 * hf + 384],
                                             start=(t == 0), stop=(t == NDT - 1))
                        nc.scalar.copy(v_sb[:, m, 384 * hf:384 * hf + 384], ps[:, 0:384])
                    return emit

                vgroups = [make_vgroup(m, hf) for hf in range(2) for m in range(NKT)]

                def emit_prep(qt):
                    # d2 = |pos_k - pos_q|^2 for this q-tile, then 9 step masks
                    dx = prep.tile([128, S], F32, tag="dx", bufs=1)
                    dy = prep.tile([128, S], F32, tag="dy", bufs=1)
                    nc.vector.tensor_scalar(out=dx[:], in0=xk_b[:], scalar1=xq_col[:, qt:qt + 1],
                                            scalar2=None, op0=ALU.subtract)
                    nc.vector.tensor_scalar(out=dy[:], in0=yk_b[:], scalar1=yq_col[:, qt:qt + 1],
                                            scalar2=None, op0=ALU.subtract)
                    dx2 = prep.tile([128, S], F32, tag="dx2", bufs=1)
                    dy2 = prep.tile([128, S], F32, tag="dy2", bufs=1)
                    nc.scalar.square(dx2[:], dx[:])
                    nc.scalar.square(dy2[:], dy[:])
                    d2 = prep.tile([128, S], F32, tag="d2", bufs=1)
                    nc.vector.tensor_add(d2[:], dx2[:], dy2[:])
                    masks = prep.tile([128, NUM_EMB - 1, S], FP16, tag="masks", bufs=2)
                    for e in range(NUM_EMB - 1):
                        nc.vector.tensor_scalar(out=masks[:, e, :], in0=d2[:],
                                                scalar1=THRESH2[e], scalar2=None,
                                                op0=ALU.is_ge)
                    return masks

                masks_by_qt = {0: emit_prep(0), 1: emit_prep(1)}
                # DVE-routed heads: bias applied via a serial STT chain on the
                # Vector engine instead of PE diag matmuls (engine balancing).
                DVE_HEADS = (9, 10, 11)

                def emit_qk(qt, h, stop):
                    off = (64 * h) % 128
                    qk = qk_ps.tile([128, S], F32, tag="qk")
                    for hf in range(2):
                        sl = slice(512 * hf, 512 * hf + 512)
                        nc.tensor.matmul(qk[:, sl],
                                         qT[off:off + 64, h // 2, 128 * qt:128 * qt + 128],
                                         kT[off:off + 64, h // 2, sl],
                                         start=True, stop=stop)
                    return qk

                def emit_exp(src):
                    # bufs=6: tails lag fronts by up to ~5 units; exp(h) must
                    # never block on a den/p buffer held by a lagging tail
                    # (ACT-queue deadlock: tail's pT-copy sits behind exp(h)).
                    p_sb = att.tile([128, S], BF16, tag="p", bufs=8)
                    den = att.tile([128, 1], F32, tag="den", bufs=8)
                    nc.scalar.activation(p_sb[:], src[:], ACT.Exp, scale=0.125,
                                         accum_out=den[:])
                    return p_sb, den

                def emit_front(qt, h, masks):
                    # DVE: 9 per-head diag(dqe) builds
                    diag = att.tile([128, NUM_EMB - 1, 128], FP16, tag="diag")
                    for e in range(NUM_EMB - 1):
                        nc.vector.tensor_scalar(out=diag[:, e, :], in0=ident16[:],
                                                scalar1=dqe[:, qt, h, e:e + 1],
                                                scalar2=None, op0=ALU.mult)
                    # PE: qk + bias into one PSUM group per 512-half
                    off = (64 * h) % 128
                    qk = qk_ps.tile([128, S], F32, tag="qk")
                    for hf in range(2):
                        sl = slice(512 * hf, 512 * hf + 512)
                        nc.tensor.matmul(qk[:, sl],
                                         qT[off:off + 64, h // 2, 128 * qt:128 * qt + 128],
                                         kT[off:off + 64, h // 2, sl],
                                         start=True, stop=False)
                        for e in range(NUM_EMB - 1):
                            nc.tensor.matmul(qk[:, sl], diag[:, e, :], masks[:, e, sl],
                                             start=False, stop=(e == NUM_EMB - 2))
                    return emit_exp(qk)

                def make_chain_ops(qt, h, masks):
                    """DVE-route front for head h: QK on PE, then 9 STT bias ops
                    on DVE (returned as thunks for interleaved emission), then exp.
                    Returns (ops, finish) where finish() emits the exp."""
                    qk = emit_qk(qt, h, stop=True)
                    accs = [att.tile([128, S], F32, tag=f"chain{i}", name=f"chain_{qt}_{h}_{i}")
                            for i in range(2)]
                    state = {"src": qk, "e": 0}

                    def op():
                        e = state["e"]
                        dst = accs[e % 2]
                        nc.vector.scalar_tensor_tensor(
                            out=dst[:], in0=masks[:, e, :], scalar=dqe[:, qt, h, e:e + 1],
                            in1=state["src"][:], op0=ALU.mult, op1=ALU.add)
                        state["src"] = dst
                        state["e"] = e + 1

                    def finish():
                        return emit_exp(state["src"])

                    return [op] * (NUM_EMB - 1), finish

                def emit_tail(qt, h, p_sb, den, o_parts):
                    # PE: transpose P to [k, q] chunks; ACT: evacuate
                    ptp = pt_ps.tile([128, NKT, 128], BF16, tag="ptp")
                    for c in range(NKT):
                        nc.tensor.transpose(ptp[:, c, :], p_sb[:, 128 * c:128 * c + 128],
                                            identb[:])
                    pT = att.tile([128, NKT, 128], BF16, tag="pT")
                    nc.scalar.copy(pT[:], ptp[:])
                    # PE: AV accumulate over k chunks
                    av = av_ps.tile([128, DK], F32, tag="av")
                    for c in range(NKT):
                        nc.tensor.matmul(av[:], pT[:, c, :], v_sb[:, c, 64 * h:64 * h + 64],
                                         start=(c == 0), stop=(c == NKT - 1))
                    # DVE: out_h = av/den + bv_h
                    recip = att.tile([128, 1], F32, tag="recip")
                    nc.vector.reciprocal(recip[:], den[:])
                    nc.vector.scalar_tensor_tensor(
                        out=o_parts[:, h, :], in0=av[:], scalar=recip[:],
                        in1=bv_b[:, 64 * h:64 * h + 64], op0=ALU.mult, op1=ALU.add)

                pe_heads = [h for h in range(H) if h not in DVE_HEADS]
                for qt in range(NQT):
                    o_parts = att.tile([128, H, DK], F32, tag="o", name=f"o_{qt}")
                    masks = masks_by_qt.pop(qt)
                    tail_q = []      # (h, p_sb, den)
                    chains = []      # [ops_list, finish, h]
                    for i, h in enumerate(pe_heads):
                        tail_q.append((h,) + emit_front(qt, h, masks))
                        # qt0 only: slip 2 V-projection groups in after each front
                        for _ in range(2):
                            if vgroups:
                                vgroups.pop(0)()
                        if i < len(DVE_HEADS):
                            ops, fin = make_chain_ops(qt, DVE_HEADS[i], masks)
                            chains.append([list(ops), fin, DVE_HEADS[i]])
                            chains[-1][0].pop(0)()  # first STT frees the qk bank
                        else:
                            # drain 4 chain STT ops, alternating between chains
                            for _ in range(4):
                                for ch in chains:
                                    if ch[0]:
                                        ch[0].pop(0)()
                                        break
                            for ch in [c for c in chains if not c[0] and c[1]]:
                                tail_q.append((ch[2],) + ch[1]())
                                ch[1] = None
                            if tail_q:
                                th, tp, td = tail_q.pop(0)
                                emit_tail(qt, th, tp, td, o_parts)

                    # flush remaining chain ops / finishes / tails
                    for ch in chains:
                        while ch[0]:
                            ch[0].pop(0)()
                        if ch[1]:
                            tail_q.append((ch[2],) + ch[1]())
                            ch[1] = None
                    for th, tp, td in tail_q:
                        emit_tail(qt, th, tp, td, o_parts)
                    if qt + 2 < NQT:
                        masks_by_qt[qt + 2] = emit_prep(qt + 2)
                    nc.sync.dma_start(out=out[128 * qt:128 * qt + 128, :],
                                      in_=o_parts[:].rearrange("p h d -> p (h d)"))
    nc.compile()
    return nc


_NC_CACHE = {}


def _get_nc():
    if "nc" not in _NC_CACHE:
        _NC_CACHE["nc"] = build_nc()
    return _NC_CACHE["nc"]


def _make_in_maps(inputs):
    q = np.asarray(inputs["query"], dtype=np.float32)
    k = np.asarray(inputs["key"], dtype=np.float32)
    v = np.asarray(inputs["value"], dtype=np.float32)
    tp = np.ascontiguousarray(np.asarray(inputs["tile_positions"], dtype=np.float32))
    f16 = lambda a: np.ascontiguousarray(a.astype(np.float16))
    wqT = f16(np.asarray(inputs["Wq"], dtype=np.float32).T)
    wkT = f16(np.asarray(inputs["Wk"], dtype=np.float32).T)
    wvT = f16(np.asarray(inputs["Wv"], dtype=np.float32).T)
    embT = f16(np.asarray(inputs["emb_k"], dtype=np.float32).T)
    bqa = np.ascontiguousarray(np.asarray(inputs["bq"], dtype=np.float32))
    bka = np.ascontiguousarray(np.asarray(inputs["bk"], dtype=np.float32))
    bva = np.ascontiguousarray(np.asarray(inputs["bv"], dtype=np.float32))
    xkT = [f16(k[b].T) for b in range(B)]
    xvT = [f16(v[b].T) for b in range(B)]
    in_maps = []
    for c in range(NCORES):
        b, qh = c // 2, c % 2
        in_maps.append({
            "xqT": f16(q[b, qh * SQ:(qh + 1) * SQ].T),
            "xkT": xkT[b], "xvT": xvT[b],
            "pos": tp[b],
            "posq": np.ascontiguousarray(tp[b, qh * SQ:(qh + 1) * SQ]),
            "wqT": wqT, "wkT": wkT, "wvT": wvT,
            "bq": bqa, "bk": bka, "bv": bva,
            "embT": embT,
        })
    return in_maps


def kernel(query, key, value, tile_positions, Wq, bq, Wk, bk, Wv, bv, emb_k):
    inputs = {"query": query, "key": key, "value": value,
              "tile_positions": tile_positions,
              "Wq": Wq, "bq": bq, "Wk": Wk, "bk": bk, "Wv": Wv, "bv": bv,
              "emb_k": emb_k}
    nc = _get_nc()
    in_maps = _make_in_maps(inputs)
    res = run_bass_kernel_spmd(nc, in_maps, core_ids=list(range(NCORES)))
    out = np.empty((B, S, D), np.float32)
    for c in range(NCORES):
        b, qh = c // 2, c % 2
        out[b, qh * SQ:(qh + 1) * SQ] = res.results[c]["out"]
    return out


# revision 14
# speedup vs baseline: 1.0226x; 1.0226x over previous
"""Distance-aware multihead attention on 8 Trainium2 NeuronCores (v3).

Problem: B=4, S=1024, D=768, H=12, DK=64, NUM_EMB=10.
  q/k/v = linear projections of query/key/value
  idx[b,i,j] = clip(round(9 * |pos_i - pos_j| / MAXD), 0, 9)
  logits = (q.k^T + qe[b,h,i,idx[b,i,j]]) / 8   where qe = q @ emb_k^T
  out = softmax(logits) @ v

Key decompositions:
  - bias qe[...,idx] = sum_{e=1..9} (qe_e - qe_{e-1}) * (d2 >= T_e^2); the
    qe_0 term is constant along the softmax axis and cancels.
  - step masks (d2 >= T_e^2) are shared across all 12 heads of a q-tile.
  - bias applied on the TENSOR engine: 9 diag(dqe_e) @ mask_e matmuls
    accumulated into the same PSUM group as the QK matmul (a diag matmul is
    a per-partition row scaling).  No per-element vector work for the bias.

v3: X and W arrive from the host already transposed ([in_dim, token] /
[in_dim, out_dim]) and cast to fp16, so DMA loads land in the exact SBUF
layout the PE needs - no on-chip transposition of inputs at all.  All PE
operands fp16 except P/V (bf16 for exp range).  Head loop is software-
pipelined (PE group for head h+1 issues before the transpose/AV tail of
head h).

Sharding: core c handles batch c//2, query-half c%2 (512 queries, all heads).
"""
import numpy as np

import concourse.bass as bass
import concourse.tile as tile
from concourse import bacc, mybir
from concourse.bass_utils import run_bass_kernel_spmd

F32 = mybir.dt.float32
BF16 = mybir.dt.bfloat16
FP16 = mybir.dt.float16
ACT = mybir.ActivationFunctionType
ALU = mybir.AluOpType

B, S, D = 4, 1024, 768
H, DK = 12, 64
NUM_EMB = 10
MAX_DIST = 100000.0 * 2 ** 0.5
SQ = S // 2          # queries per core
NQT = SQ // 128      # q-tiles per core (4)
NKT = S // 128       # k token chunks (8)
NDT = D // 128       # dim tiles (6)
NCORES = 8

# squared thresholds: idx >= e  <=>  d2 >= ((e-0.5)*MAX_DIST/9)^2
THRESH2 = [float(((e - 0.5) * MAX_DIST / 9.0) ** 2) for e in range(1, NUM_EMB)]


def build_nc():
    nc = bacc.Bacc("TRN2", target_bir_lowering=False, debug=False)

    # host-pretransposed fp16 operands: [contraction_dim, free_dim]
    xqT_d = nc.dram_tensor("xqT", [D, SQ], FP16, kind="ExternalInput").ap()
    xkT_d = nc.dram_tensor("xkT", [D, S], FP16, kind="ExternalInput").ap()
    xvT_d = nc.dram_tensor("xvT", [D, S], FP16, kind="ExternalInput").ap()
    wqT_d = nc.dram_tensor("wqT", [D, D], FP16, kind="ExternalInput").ap()
    wkT_d = nc.dram_tensor("wkT", [D, D], FP16, kind="ExternalInput").ap()
    wvT_d = nc.dram_tensor("wvT", [D, D], FP16, kind="ExternalInput").ap()
    embT_d = nc.dram_tensor("embT", [DK, NUM_EMB], FP16, kind="ExternalInput").ap()
    pos = nc.dram_tensor("pos", [S, 2], F32, kind="ExternalInput").ap()
    posq = nc.dram_tensor("posq", [SQ, 2], F32, kind="ExternalInput").ap()
    bq = nc.dram_tensor("bq", [D], F32, kind="ExternalInput").ap()
    bk = nc.dram_tensor("bk", [D], F32, kind="ExternalInput").ap()
    bv = nc.dram_tensor("bv", [D], F32, kind="ExternalInput").ap()
    out = nc.dram_tensor("out", [SQ, D], F32, kind="ExternalOutput").ap()

    with tile.TileContext(nc) as tc:
        with tc.tile_pool(name="persist", bufs=1) as persist:
            from concourse.masks import make_identity
            ident16 = persist.tile([128, 128], FP16)
            identb = persist.tile([128, 128], BF16)
            make_identity(nc, ident16[:])
            make_identity(nc, identb[:])

            # small loads on the Pool SWDGE queue
            bq_col = persist.tile([128, NDT], F32)
            bk_col = persist.tile([128, NDT], F32)
            nc.gpsimd.dma_start(out=bq_col[:], in_=bass.AP(tensor=bq.tensor, offset=0, ap=[[1, 128], [128, NDT]]))
            nc.gpsimd.dma_start(out=bk_col[:], in_=bass.AP(tensor=bk.tensor, offset=0, ap=[[1, 128], [128, NDT]]))
            bv_b = persist.tile([128, D], F32)
            nc.gpsimd.dma_start(out=bv_b[:], in_=bass.AP(tensor=bv.tensor, offset=0, ap=[[0, 128], [1, D]]))
            # position x/y as single-partition rows [1, S]
            posx_row = persist.tile([1, S], F32)
            posy_row = persist.tile([1, S], F32)
            nc.gpsimd.dma_start(out=posx_row[:], in_=bass.AP(tensor=pos.tensor, offset=0, ap=[[2, 1], [2, S]]))
            nc.gpsimd.dma_start(out=posy_row[:], in_=bass.AP(tensor=pos.tensor, offset=1, ap=[[2, 1], [2, S]]))
            # query positions as per-partition scalars [128, NQT]
            xq_col = persist.tile([128, NQT], F32)
            yq_col = persist.tile([128, NQT], F32)
            nc.gpsimd.dma_start(out=xq_col[:], in_=bass.AP(tensor=posq.tensor, offset=0, ap=[[2, 128], [256, NQT]]))
            nc.gpsimd.dma_start(out=yq_col[:], in_=bass.AP(tensor=posq.tensor, offset=1, ap=[[2, 128], [256, NQT]]))
            # emb^T block-diagonal [128, 2*NUM_EMB] fp16 (2 heads per matmul)
            embT_blk = persist.tile([128, 2 * NUM_EMB], FP16)
            nc.vector.memset(embT_blk[:], 0.0)
            nc.gpsimd.dma_start(out=embT_blk[0:64, 0:NUM_EMB], in_=embT_d[:, :])
            nc.gpsimd.dma_start(out=embT_blk[64:128, NUM_EMB:2 * NUM_EMB], in_=embT_d[:, :])

            ones1 = persist.tile([1, 128], F32)
            nc.vector.memset(ones1[:], 1.0)

            # persistent attention operands
            kT = persist.tile([128, NDT, S], FP16)      # K^T[dim, token] + bk
            qT = persist.tile([128, NDT, SQ], FP16)     # Q^T[dim, token] + bq
            v_sb = persist.tile([128, NKT, D], BF16)    # V[token, dim] (no bias)
            xk_b = persist.tile([128, S], F32)          # pos-x broadcast rows
            yk_b = persist.tile([128, S], F32)
            dqe = persist.tile([128, NQT, H, NUM_EMB - 1], F32)

            # ---- load (already transposed), project, attention ----
            # PSUM budget (8 banks): pj 2 + qk 4 + ptp 1 + av 1
            with tc.tile_pool(name="tsp", bufs=2) as tsp, \
                 tc.tile_pool(name="pj_ps", bufs=2, space="PSUM") as pj_ps, \
                 tc.tile_pool(name="att", bufs=2) as att, \
                 tc.tile_pool(name="prep", bufs=2) as prep, \
                 tc.tile_pool(name="qk_ps", bufs=2, space="PSUM") as qk_ps, \
                 tc.tile_pool(name="pt_ps", bufs=1, space="PSUM") as pt_ps, \
                 tc.tile_pool(name="av_ps", bufs=1, space="PSUM") as av_ps:

                # broadcast pos rows across partitions via 1-partition matmul
                for dst, row in ((xk_b, posx_row), (yk_b, posy_row)):
                    for hf in range(2):
                        sl = slice(512 * hf, 512 * hf + 512)
                        bc = pj_ps.tile([128, 512], F32, tag="pj")
                        nc.tensor.matmul(bc[:], ones1[:], row[:, sl],
                                         start=True, stop=True)
                        nc.scalar.copy(dst[:, sl], bc[:])

                def load_T(src, dst, ncols, dma_eng):
                    for d in range(NDT):
                        dma_eng.dma_start(out=dst[:, d, 0:ncols],
                                          in_=src[128 * d:128 * (d + 1), :])

                # Q first: attention fronts need qT/dqe earliest
                xqT = tsp.tile([128, NDT, S], FP16, tag="xT")
                wqT = tsp.tile([128, NDT, D], FP16, tag="wT")
                load_T(xqT_d, xqT, SQ, nc.sync)
                load_T(wqT_d, wqT, D, nc.scalar)
                for m in range(NDT):
                    ps = pj_ps.tile([128, 512], F32, tag="pj")
                    for t in range(NDT):
                        nc.tensor.matmul(ps[:], wqT[:, t, 128 * m:128 * m + 128],
                                         xqT[:, t, 0:SQ],
                                         start=(t == 0), stop=(t == NDT - 1))
                    nc.scalar.activation(qT[:, m, :], ps[:], ACT.Identity,
                                         bias=bq_col[:, m:m + 1])

                # qe for all q-tiles: block-diag emb matmul, 2 heads per 128-dim block
                qe_psum = pj_ps.tile([128, 512], F32, tag="pj")
                for qt in range(NQT):
                    for m in range(NDT):
                        nc.tensor.matmul(qe_psum[:, 120 * qt + 20 * m:120 * qt + 20 * m + 20],
                                         qT[:, m, 128 * qt:128 * qt + 128],
                                         embT_blk[:],
                                         start=True, stop=True)
                qe_sb = persist.tile([128, NQT, H, NUM_EMB], F32)
                nc.scalar.copy(qe_sb[:], qe_psum[:, 0:NQT * H * NUM_EMB]
                               .rearrange("p (q h e) -> p q h e", e=NUM_EMB, h=H))
                nc.vector.tensor_tensor(out=dqe[:], in0=qe_sb[:, :, :, 1:],
                                        in1=qe_sb[:, :, :, :-1], op=ALU.subtract)

                # K
                xkT = tsp.tile([128, NDT, S], FP16, tag="xT")
                wkT = tsp.tile([128, NDT, D], FP16, tag="wT")
                load_T(xkT_d, xkT, S, nc.sync)
                load_T(wkT_d, wkT, D, nc.scalar)
                for m in range(NDT):
                    for hf in range(2):
                        ps = pj_ps.tile([128, 512], F32, tag="pj")
                        for t in range(NDT):
                            nc.tensor.matmul(ps[:], wkT[:, t, 128 * m:128 * m + 128],
                                             xkT[:, t, 512 * hf:512 * hf + 512],
                                             start=(t == 0), stop=(t == NDT - 1))
                        nc.scalar.activation(kT[:, m, 512 * hf:512 * hf + 512], ps[:],
                                             ACT.Identity, bias=bk_col[:, m:m + 1])

                # V: loads issued now; the 16 projection groups are emitted
                # interleaved into qt0's head loop (PE work hidden under attention)
                xvT = tsp.tile([128, NDT, S], FP16, tag="xT")
                wvT = tsp.tile([128, NDT, D], FP16, tag="wT")
                load_T(xvT_d, xvT, S, nc.sync)
                load_T(wvT_d, wvT, D, nc.scalar)

                def make_vgroup(m, hf):
                    def emit():
                        ps = pj_ps.tile([128, 512], F32, tag="pj")
                        for t in range(NDT):
                            nc.tensor.matmul(ps[:, 0:384], xvT[:, t, 128 * m:128 * m + 128],
                                             wvT[:, t, 384 * hf:384 * hf + 384],
                                             start=(t == 0), stop=(t == NDT - 1))
                        nc.scalar.copy(v_sb[:, m, 384 * hf:384 * hf + 384], ps[:, 0:384])
                    return emit

                vgroups = [make_vgroup(m, hf) for hf in range(2) for m in range(NKT)]

                def emit_prep(qt):
                    # d2 = |pos_k - pos_q|^2 for this q-tile, then 9 step masks
                    dx = prep.tile([128, S], F32, tag="dx", bufs=1)
                    dy = prep.tile([128, S], F32, tag="dy", bufs=1)
                    nc.vector.tensor_scalar(out=dx[:], in0=xk_b[:], scalar1=xq_col[:, qt:qt + 1],
                                            scalar2=None, op0=ALU.subtract)
                    nc.vector.tensor_scalar(out=dy[:], in0=yk_b[:], scalar1=yq_col[:, qt:qt + 1],
                                            scalar2=None, op0=ALU.subtract)
                    dx2 = prep.tile([128, S], F32, tag="dx2", bufs=1)
                    dy2 = prep.tile([128, S], F32, tag="dy2", bufs=1)
                    nc.scalar.square(dx2[:], dx[:])
                    nc.scalar.square(dy2[:], dy[:])
                    d2 = prep.tile([128, S], F32, tag="d2", bufs=1)
                    nc.vector.tensor_add(d2[:], dx2[:], dy2[:])
                    masks = prep.tile([128, NUM_EMB - 1, S], FP16, tag="masks", bufs=2)
                    for e in range(NUM_EMB - 1):
                        nc.vector.tensor_scalar(out=masks[:, e, :], in0=d2[:],
                                                scalar1=THRESH2[e], scalar2=None,
                                                op0=ALU.is_ge)
                    return masks

                masks_by_qt = {0: emit_prep(0), 1: emit_prep(1)}
                # DVE-routed heads: bias applied via a serial STT chain on the
                # Vector engine instead of PE diag matmuls (engine balancing;
                # alternating 3/2 per q-tile gives an effective 2.5 split).
                def dve_heads(qt):
                    return (9, 10, 11) if qt % 2 == 0 else (10, 11)

                def emit_qk(qt, h, stop):
                    off = (64 * h) % 128
                    qk = qk_ps.tile([128, S], F32, tag="qk")
                    for hf in range(2):
                        sl = slice(512 * hf, 512 * hf + 512)
                        nc.tensor.matmul(qk[:, sl],
                                         qT[off:off + 64, h // 2, 128 * qt:128 * qt + 128],
                                         kT[off:off + 64, h // 2, sl],
                                         start=True, stop=stop)
                    return qk

                def emit_exp(src):
                    # bufs=6: tails lag fronts by up to ~5 units; exp(h) must
                    # never block on a den/p buffer held by a lagging tail
                    # (ACT-queue deadlock: tail's pT-copy sits behind exp(h)).
                    p_sb = att.tile([128, S], BF16, tag="p", bufs=8)
                    den = att.tile([128, 1], F32, tag="den", bufs=8)
                    nc.scalar.activation(p_sb[:], src[:], ACT.Exp, scale=0.125,
                                         accum_out=den[:])
                    return p_sb, den

                def emit_front(qt, h, masks):
                    # DVE: 9 per-head diag(dqe) builds
                    diag = att.tile([128, NUM_EMB - 1, 128], FP16, tag="diag")
                    for e in range(NUM_EMB - 1):
                        nc.vector.tensor_scalar(out=diag[:, e, :], in0=ident16[:],
                                                scalar1=dqe[:, qt, h, e:e + 1],
                                                scalar2=None, op0=ALU.mult)
                    # PE: qk + bias into one PSUM group per 512-half
                    off = (64 * h) % 128
                    qk = qk_ps.tile([128, S], F32, tag="qk")
                    for hf in range(2):
                        sl = slice(512 * hf, 512 * hf + 512)
                        nc.tensor.matmul(qk[:, sl],
                                         qT[off:off + 64, h // 2, 128 * qt:128 * qt + 128],
                                         kT[off:off + 64, h // 2, sl],
                                         start=True, stop=False)
                        for e in range(NUM_EMB - 1):
                            nc.tensor.matmul(qk[:, sl], diag[:, e, :], masks[:, e, sl],
                                             start=False, stop=(e == NUM_EMB - 2))
                    return emit_exp(qk)

                def make_chain_ops(qt, h, masks):
                    """DVE-route front for head h: QK on PE, then 9 STT bias ops
                    on DVE (returned as thunks for interleaved emission), then exp.
                    Returns (ops, finish) where finish() emits the exp."""
                    qk = emit_qk(qt, h, stop=True)
                    accs = [att.tile([128, S], F32, tag=f"chain{i}", name=f"chain_{qt}_{h}_{i}")
                            for i in range(2)]
                    state = {"src": qk, "e": 0}

                    def op():
                        e = state["e"]
                        dst = accs[e % 2]
                        nc.vector.scalar_tensor_tensor(
                            out=dst[:], in0=masks[:, e, :], scalar=dqe[:, qt, h, e:e + 1],
                            in1=state["src"][:], op0=ALU.mult, op1=ALU.add)
                        state["src"] = dst
                        state["e"] = e + 1

                    def finish():
                        return emit_exp(state["src"])

                    return [op] * (NUM_EMB - 1), finish

                def emit_tail(qt, h, p_sb, den, o_parts):
                    # PE: transpose P to [k, q] chunks; ACT: evacuate
                    ptp = pt_ps.tile([128, NKT, 128], BF16, tag="ptp")
                    for c in range(NKT):
                        nc.tensor.transpose(ptp[:, c, :], p_sb[:, 128 * c:128 * c + 128],
                                            identb[:])
                    pT = att.tile([128, NKT, 128], BF16, tag="pT")
                    nc.scalar.copy(pT[:], ptp[:])
                    # PE: AV accumulate over k chunks
                    av = av_ps.tile([128, DK], F32, tag="av")
                    for c in range(NKT):
                        nc.tensor.matmul(av[:], pT[:, c, :], v_sb[:, c, 64 * h:64 * h + 64],
                                         start=(c == 0), stop=(c == NKT - 1))
                    # DVE: out_h = av/den + bv_h
                    recip = att.tile([128, 1], F32, tag="recip")
                    nc.vector.reciprocal(recip[:], den[:])
                    nc.vector.scalar_tensor_tensor(
                        out=o_parts[:, h, :], in0=av[:], scalar=recip[:],
                        in1=bv_b[:, 64 * h:64 * h + 64], op0=ALU.mult, op1=ALU.add)

                for qt in range(NQT):
                    DVE_HEADS = dve_heads(qt)
                    pe_heads = [h for h in range(H) if h not in DVE_HEADS]
                    o_parts = att.tile([128, H, DK], F32, tag="o", name=f"o_{qt}")
                    masks = masks_by_qt.pop(qt)
                    tail_q = []      # (h, p_sb, den)
                    chains = []      # [ops_list, finish, h]
                    for i, h in enumerate(pe_heads):
                        tail_q.append((h,) + emit_front(qt, h, masks))
                        # qt0 only: slip 2 V-projection groups in after each front
                        for _ in range(2):
                            if vgroups:
                                vgroups.pop(0)()
                        if i < len(DVE_HEADS):
                            ops, fin = make_chain_ops(qt, DVE_HEADS[i], masks)
                            chains.append([list(ops), fin, DVE_HEADS[i]])
                            chains[-1][0].pop(0)()  # first STT frees the qk bank
                        else:
                            # drain chain STT ops, alternating between chains
                            nslots = len(pe_heads) - len(DVE_HEADS)
                            ndrain = -(-(8 * len(DVE_HEADS)) // nslots)
                            for _ in range(ndrain):
                                for ch in chains:
                                    if ch[0]:
                                        ch[0].pop(0)()
                                        break
                            for ch in [c for c in chains if not c[0] and c[1]]:
                                tail_q.append((ch[2],) + ch[1]())
                                ch[1] = None
                            for _ in range(2 if i >= 6 else 1):
                                if tail_q:
                                    th, tp, td = tail_q.pop(0)
                                    emit_tail(qt, th, tp, td, o_parts)

                    # flush remaining chain ops / finishes / tails
                    for ch in chains:
                        while ch[0]:
                            ch[0].pop(0)()
                        if ch[1]:
                            tail_q.append((ch[2],) + ch[1]())
                            ch[1] = None
                    for th, tp, td in tail_q:
                        emit_tail(qt, th, tp, td, o_parts)
                    if qt + 2 < NQT:
                        masks_by_qt[qt + 2] = emit_prep(qt + 2)
                    nc.sync.dma_start(out=out[128 * qt:128 * qt + 128, :],
                                      in_=o_parts[:].rearrange("p h d -> p (h d)"))
    nc.compile()
    return nc


_NC_CACHE = {}


def _get_nc():
    if "nc" not in _NC_CACHE:
        _NC_CACHE["nc"] = build_nc()
    return _NC_CACHE["nc"]


def _make_in_maps(inputs):
    q = np.asarray(inputs["query"], dtype=np.float32)
    k = np.asarray(inputs["key"], dtype=np.float32)
    v = np.asarray(inputs["value"], dtype=np.float32)
    tp = np.ascontiguousarray(np.asarray(inputs["tile_positions"], dtype=np.float32))
    f16 = lambda a: np.ascontiguousarray(a.astype(np.float16))
    wqT = f16(np.asarray(inputs["Wq"], dtype=np.float32).T)
    wkT = f16(np.asarray(inputs["Wk"], dtype=np.float32).T)
    wvT = f16(np.asarray(inputs["Wv"], dtype=np.float32).T)
    embT = f16(np.asarray(inputs["emb_k"], dtype=np.float32).T)
    bqa = np.ascontiguousarray(np.asarray(inputs["bq"], dtype=np.float32))
    bka = np.ascontiguousarray(np.asarray(inputs["bk"], dtype=np.float32))
    bva = np.ascontiguousarray(np.asarray(inputs["bv"], dtype=np.float32))
    xkT = [f16(k[b].T) for b in range(B)]
    xvT = [f16(v[b].T) for b in range(B)]
    in_maps = []
    for c in range(NCORES):
        b, qh = c // 2, c % 2
        in_maps.append({
            "xqT": f16(q[b, qh * SQ:(qh + 1) * SQ].T),
            "xkT": xkT[b], "xvT": xvT[b],
            "pos": tp[b],
            "posq": np.ascontiguousarray(tp[b, qh * SQ:(qh + 1) * SQ]),
            "wqT": wqT, "wkT": wkT, "wvT": wvT,
            "bq": bqa, "bk": bka, "bv": bva,
            "embT": embT,
        })
    return in_maps


def kernel(query, key, value, tile_positions, Wq, bq, Wk, bk, Wv, bv, emb_k):
    inputs = {"query": query, "key": key, "value": value,
              "tile_positions": tile_positions,
              "Wq": Wq, "bq": bq, "Wk": Wk, "bk": bk, "Wv": Wv, "bv": bv,
              "emb_k": emb_k}
    nc = _get_nc()
    in_maps = _make_in_maps(inputs)
    res = run_bass_kernel_spmd(nc, in_maps, core_ids=list(range(NCORES)))
    out = np.empty((B, S, D), np.float32)
    for c in range(NCORES):
        b, qh = c // 2, c % 2
        out[b, qh * SQ:(qh + 1) * SQ] = res.results[c]["out"]
    return out


# revision 15
# speedup vs baseline: 1.0479x; 1.0248x over previous
"""Distance-aware multihead attention on 8 Trainium2 NeuronCores (v3).

Problem: B=4, S=1024, D=768, H=12, DK=64, NUM_EMB=10.
  q/k/v = linear projections of query/key/value
  idx[b,i,j] = clip(round(9 * |pos_i - pos_j| / MAXD), 0, 9)
  logits = (q.k^T + qe[b,h,i,idx[b,i,j]]) / 8   where qe = q @ emb_k^T
  out = softmax(logits) @ v

Key decompositions:
  - bias qe[...,idx] = sum_{e=1..9} (qe_e - qe_{e-1}) * (d2 >= T_e^2); the
    qe_0 term is constant along the softmax axis and cancels.
  - step masks (d2 >= T_e^2) are shared across all 12 heads of a q-tile.
  - bias applied on the TENSOR engine: 9 diag(dqe_e) @ mask_e matmuls
    accumulated into the same PSUM group as the QK matmul (a diag matmul is
    a per-partition row scaling).  No per-element vector work for the bias.

v3: X and W arrive from the host already transposed ([in_dim, token] /
[in_dim, out_dim]) and cast to fp16, so DMA loads land in the exact SBUF
layout the PE needs - no on-chip transposition of inputs at all.  All PE
operands fp16 except P/V (bf16 for exp range).  Head loop is software-
pipelined (PE group for head h+1 issues before the transpose/AV tail of
head h).

Sharding: core c handles batch c//2, query-half c%2 (512 queries, all heads).
"""
import numpy as np

import concourse.bass as bass
import concourse.tile as tile
from concourse import bacc, mybir
from concourse.bass_utils import run_bass_kernel_spmd

F32 = mybir.dt.float32
BF16 = mybir.dt.bfloat16
FP16 = mybir.dt.float16
ACT = mybir.ActivationFunctionType
ALU = mybir.AluOpType

B, S, D = 4, 1024, 768
H, DK = 12, 64
NUM_EMB = 10
MAX_DIST = 100000.0 * 2 ** 0.5
SQ = S // 2          # queries per core
NQT = SQ // 128      # q-tiles per core (4)
NKT = S // 128       # k token chunks (8)
NDT = D // 128       # dim tiles (6)
NCORES = 8

# squared thresholds: idx >= e  <=>  d2 >= ((e-0.5)*MAX_DIST/9)^2
THRESH2 = [float(((e - 0.5) * MAX_DIST / 9.0) ** 2) for e in range(1, NUM_EMB)]


def build_nc():
    nc = bacc.Bacc("TRN2", target_bir_lowering=False, debug=False)

    # host-pretransposed fp16 operands: [contraction_dim, free_dim]
    xqT_d = nc.dram_tensor("xqT", [D, SQ], FP16, kind="ExternalInput").ap()
    xkT_d = nc.dram_tensor("xkT", [D, S], FP16, kind="ExternalInput").ap()
    xvT_d = nc.dram_tensor("xvT", [D, S], FP16, kind="ExternalInput").ap()
    wqT_d = nc.dram_tensor("wqT", [D, D], FP16, kind="ExternalInput").ap()
    wkT_d = nc.dram_tensor("wkT", [D, D], FP16, kind="ExternalInput").ap()
    wvT_d = nc.dram_tensor("wvT", [D, D], FP16, kind="ExternalInput").ap()
    embT_d = nc.dram_tensor("embT", [DK, NUM_EMB], FP16, kind="ExternalInput").ap()
    pos = nc.dram_tensor("pos", [S, 2], F32, kind="ExternalInput").ap()
    posq = nc.dram_tensor("posq", [SQ, 2], F32, kind="ExternalInput").ap()
    bq = nc.dram_tensor("bq", [D], F32, kind="ExternalInput").ap()
    bk = nc.dram_tensor("bk", [D], F32, kind="ExternalInput").ap()
    bv = nc.dram_tensor("bv", [D], F32, kind="ExternalInput").ap()
    out = nc.dram_tensor("out", [SQ, D], F32, kind="ExternalOutput").ap()

    with tile.TileContext(nc) as tc:
        with tc.tile_pool(name="persist", bufs=1) as persist:
            from concourse.masks import make_identity
            ident16 = persist.tile([128, 128], FP16)
            identb = persist.tile([128, 128], BF16)
            make_identity(nc, ident16[:])
            make_identity(nc, identb[:])

            # small loads on the Pool SWDGE queue
            bq_col = persist.tile([128, NDT], F32)
            bk_col = persist.tile([128, NDT], F32)
            nc.gpsimd.dma_start(out=bq_col[:], in_=bass.AP(tensor=bq.tensor, offset=0, ap=[[1, 128], [128, NDT]]))
            nc.gpsimd.dma_start(out=bk_col[:], in_=bass.AP(tensor=bk.tensor, offset=0, ap=[[1, 128], [128, NDT]]))
            bv_b = persist.tile([128, D], F32)
            nc.gpsimd.dma_start(out=bv_b[:], in_=bass.AP(tensor=bv.tensor, offset=0, ap=[[0, 128], [1, D]]))
            # position x/y as single-partition rows [1, S] - on the SP HWDGE
            # queue (the Pool SWDGE queue starts ~15us late; the pos broadcast
            # gates mask prep and was the head-of-kernel critical path)
            posx_row = persist.tile([1, S], F32)
            posy_row = persist.tile([1, S], F32)
            nc.sync.dma_start(out=posx_row[:], in_=bass.AP(tensor=pos.tensor, offset=0, ap=[[2, 1], [2, S]]))
            nc.sync.dma_start(out=posy_row[:], in_=bass.AP(tensor=pos.tensor, offset=1, ap=[[2, 1], [2, S]]))
            # query positions as per-partition scalars [128, NQT]
            xq_col = persist.tile([128, NQT], F32)
            yq_col = persist.tile([128, NQT], F32)
            nc.gpsimd.dma_start(out=xq_col[:], in_=bass.AP(tensor=posq.tensor, offset=0, ap=[[2, 128], [256, NQT]]))
            nc.gpsimd.dma_start(out=yq_col[:], in_=bass.AP(tensor=posq.tensor, offset=1, ap=[[2, 128], [256, NQT]]))
            # emb^T block-diagonal [128, 2*NUM_EMB] fp16 (2 heads per matmul)
            embT_blk = persist.tile([128, 2 * NUM_EMB], FP16)
            nc.vector.memset(embT_blk[:], 0.0)
            nc.gpsimd.dma_start(out=embT_blk[0:64, 0:NUM_EMB], in_=embT_d[:, :])
            nc.gpsimd.dma_start(out=embT_blk[64:128, NUM_EMB:2 * NUM_EMB], in_=embT_d[:, :])

            ones1 = persist.tile([1, 128], F32)
            nc.vector.memset(ones1[:], 1.0)

            # persistent attention operands
            kT = persist.tile([128, NDT, S], FP16)      # K^T[dim, token] + bk
            qT = persist.tile([128, NDT, SQ], FP16)     # Q^T[dim, token] + bq
            v_sb = persist.tile([128, NKT, D], BF16)    # V[token, dim] (no bias)
            xk_b = persist.tile([128, S], F32)          # pos-x broadcast rows
            yk_b = persist.tile([128, S], F32)
            dqe = persist.tile([128, NQT, H, NUM_EMB - 1], F32)

            # ---- load (already transposed), project, attention ----
            # PSUM budget (8 banks): pj 2 + qk 4 + ptp 1 + av 1
            with tc.tile_pool(name="tsp", bufs=2) as tsp, \
                 tc.tile_pool(name="pj_ps", bufs=2, space="PSUM") as pj_ps, \
                 tc.tile_pool(name="att", bufs=2) as att, \
                 tc.tile_pool(name="prep", bufs=2) as prep, \
                 tc.tile_pool(name="qk_ps", bufs=2, space="PSUM") as qk_ps, \
                 tc.tile_pool(name="pt_ps", bufs=1, space="PSUM") as pt_ps, \
                 tc.tile_pool(name="av_ps", bufs=1, space="PSUM") as av_ps:

                # broadcast pos rows across partitions via 1-partition matmul
                for dst, row in ((xk_b, posx_row), (yk_b, posy_row)):
                    for hf in range(2):
                        sl = slice(512 * hf, 512 * hf + 512)
                        bc = pj_ps.tile([128, 512], F32, tag="pj")
                        nc.tensor.matmul(bc[:], ones1[:], row[:, sl],
                                         start=True, stop=True)
                        nc.scalar.copy(dst[:, sl], bc[:])

                def load_T(src, dst, ncols, dma_eng):
                    for d in range(NDT):
                        dma_eng.dma_start(out=dst[:, d, 0:ncols],
                                          in_=src[128 * d:128 * (d + 1), :])

                # Q first: attention fronts need qT/dqe earliest
                xqT = tsp.tile([128, NDT, S], FP16, tag="xT")
                wqT = tsp.tile([128, NDT, D], FP16, tag="wT")
                load_T(xqT_d, xqT, SQ, nc.sync)
                load_T(wqT_d, wqT, D, nc.scalar)
                for m in range(NDT):
                    ps = pj_ps.tile([128, 512], F32, tag="pj")
                    for t in range(NDT):
                        nc.tensor.matmul(ps[:], wqT[:, t, 128 * m:128 * m + 128],
                                         xqT[:, t, 0:SQ],
                                         start=(t == 0), stop=(t == NDT - 1))
                    nc.scalar.activation(qT[:, m, :], ps[:], ACT.Identity,
                                         bias=bq_col[:, m:m + 1])

                # qe for all q-tiles: block-diag emb matmul, 2 heads per 128-dim block
                qe_psum = pj_ps.tile([128, 512], F32, tag="pj")
                for qt in range(NQT):
                    for m in range(NDT):
                        nc.tensor.matmul(qe_psum[:, 120 * qt + 20 * m:120 * qt + 20 * m + 20],
                                         qT[:, m, 128 * qt:128 * qt + 128],
                                         embT_blk[:],
                                         start=True, stop=True)
                qe_sb = persist.tile([128, NQT, H, NUM_EMB], F32)
                nc.scalar.copy(qe_sb[:], qe_psum[:, 0:NQT * H * NUM_EMB]
                               .rearrange("p (q h e) -> p q h e", e=NUM_EMB, h=H))
                nc.vector.tensor_tensor(out=dqe[:], in0=qe_sb[:, :, :, 1:],
                                        in1=qe_sb[:, :, :, :-1], op=ALU.subtract)

                # K
                xkT = tsp.tile([128, NDT, S], FP16, tag="xT")
                wkT = tsp.tile([128, NDT, D], FP16, tag="wT")
                load_T(xkT_d, xkT, S, nc.sync)
                load_T(wkT_d, wkT, D, nc.scalar)
                for m in range(NDT):
                    for hf in range(2):
                        ps = pj_ps.tile([128, 512], F32, tag="pj")
                        for t in range(NDT):
                            nc.tensor.matmul(ps[:], wkT[:, t, 128 * m:128 * m + 128],
                                             xkT[:, t, 512 * hf:512 * hf + 512],
                                             start=(t == 0), stop=(t == NDT - 1))
                        nc.scalar.activation(kT[:, m, 512 * hf:512 * hf + 512], ps[:],
                                             ACT.Identity, bias=bk_col[:, m:m + 1])

                # V: loads issued now; the 16 projection groups are emitted
                # interleaved into qt0's head loop (PE work hidden under attention)
                xvT = tsp.tile([128, NDT, S], FP16, tag="xT")
                wvT = tsp.tile([128, NDT, D], FP16, tag="wT")
                load_T(xvT_d, xvT, S, nc.sync)
                load_T(wvT_d, wvT, D, nc.scalar)

                def make_vgroup(m, hf):
                    def emit():
                        ps = pj_ps.tile([128, 512], F32, tag="pj")
                        for t in range(NDT):
                            nc.tensor.matmul(ps[:, 0:384], xvT[:, t, 128 * m:128 * m + 128],
                                             wvT[:, t, 384 * hf:384 * hf + 384],
                                             start=(t == 0), stop=(t == NDT - 1))
                        nc.scalar.copy(v_sb[:, m, 384 * hf:384 * hf + 384], ps[:, 0:384])
                    return emit

                vgroups = [make_vgroup(m, hf) for hf in range(2) for m in range(NKT)]

                def emit_prep(qt):
                    # d2 = |pos_k - pos_q|^2 for this q-tile, then 9 step masks
                    dx = prep.tile([128, S], F32, tag="dx", bufs=1)
                    dy = prep.tile([128, S], F32, tag="dy", bufs=1)
                    nc.vector.tensor_scalar(out=dx[:], in0=xk_b[:], scalar1=xq_col[:, qt:qt + 1],
                                            scalar2=None, op0=ALU.subtract)
                    nc.vector.tensor_scalar(out=dy[:], in0=yk_b[:], scalar1=yq_col[:, qt:qt + 1],
                                            scalar2=None, op0=ALU.subtract)
                    dx2 = prep.tile([128, S], F32, tag="dx2", bufs=1)
                    dy2 = prep.tile([128, S], F32, tag="dy2", bufs=1)
                    nc.scalar.square(dx2[:], dx[:])
                    nc.scalar.square(dy2[:], dy[:])
                    d2 = prep.tile([128, S], F32, tag="d2", bufs=1)
                    nc.vector.tensor_add(d2[:], dx2[:], dy2[:])
                    masks = prep.tile([128, NUM_EMB - 1, S], FP16, tag="masks", bufs=2)
                    for e in range(NUM_EMB - 1):
                        nc.vector.tensor_scalar(out=masks[:, e, :], in0=d2[:],
                                                scalar1=THRESH2[e], scalar2=None,
                                                op0=ALU.is_ge)
                    return masks

                masks_by_qt = {0: emit_prep(0), 1: emit_prep(1)}
                # DVE-routed heads: bias applied via a serial STT chain on the
                # Vector engine instead of PE diag matmuls (engine balancing;
                # alternating 3/2 per q-tile gives an effective 2.5 split).
                def dve_heads(qt):
                    return (9, 10, 11) if qt % 2 == 0 else (10, 11)

                def emit_qk(qt, h, stop):
                    off = (64 * h) % 128
                    qk = qk_ps.tile([128, S], F32, tag="qk")
                    for hf in range(2):
                        sl = slice(512 * hf, 512 * hf + 512)
                        nc.tensor.matmul(qk[:, sl],
                                         qT[off:off + 64, h // 2, 128 * qt:128 * qt + 128],
                                         kT[off:off + 64, h // 2, sl],
                                         start=True, stop=stop)
                    return qk

                def emit_exp(src):
                    # bufs=6: tails lag fronts by up to ~5 units; exp(h) must
                    # never block on a den/p buffer held by a lagging tail
                    # (ACT-queue deadlock: tail's pT-copy sits behind exp(h)).
                    p_sb = att.tile([128, S], BF16, tag="p", bufs=8)
                    den = att.tile([128, 1], F32, tag="den", bufs=8)
                    nc.scalar.activation(p_sb[:], src[:], ACT.Exp, scale=0.125,
                                         accum_out=den[:])
                    return p_sb, den

                def emit_front(qt, h, masks):
                    # DVE: 9 per-head diag(dqe) builds
                    diag = att.tile([128, NUM_EMB - 1, 128], FP16, tag="diag")
                    for e in range(NUM_EMB - 1):
                        nc.vector.tensor_scalar(out=diag[:, e, :], in0=ident16[:],
                                                scalar1=dqe[:, qt, h, e:e + 1],
                                                scalar2=None, op0=ALU.mult)
                    # PE: qk + bias into one PSUM group per 512-half
                    off = (64 * h) % 128
                    qk = qk_ps.tile([128, S], F32, tag="qk")
                    for hf in range(2):
                        sl = slice(512 * hf, 512 * hf + 512)
                        nc.tensor.matmul(qk[:, sl],
                                         qT[off:off + 64, h // 2, 128 * qt:128 * qt + 128],
                                         kT[off:off + 64, h // 2, sl],
                                         start=True, stop=False)
                        for e in range(NUM_EMB - 1):
                            nc.tensor.matmul(qk[:, sl], diag[:, e, :], masks[:, e, sl],
                                             start=False, stop=(e == NUM_EMB - 2))
                    return emit_exp(qk)

                def make_chain_ops(qt, h, masks):
                    """DVE-route front for head h: QK on PE, then 9 STT bias ops
                    on DVE (returned as thunks for interleaved emission), then exp.
                    Returns (ops, finish) where finish() emits the exp."""
                    qk = emit_qk(qt, h, stop=True)
                    accs = [att.tile([128, S], F32, tag=f"chain{i}", name=f"chain_{qt}_{h}_{i}")
                            for i in range(2)]
                    state = {"src": qk, "e": 0}

                    def op():
                        e = state["e"]
                        dst = accs[e % 2]
                        nc.vector.scalar_tensor_tensor(
                            out=dst[:], in0=masks[:, e, :], scalar=dqe[:, qt, h, e:e + 1],
                            in1=state["src"][:], op0=ALU.mult, op1=ALU.add)
                        state["src"] = dst
                        state["e"] = e + 1

                    def finish():
                        return emit_exp(state["src"])

                    return [op] * (NUM_EMB - 1), finish

                def emit_tail(qt, h, p_sb, den, o_parts):
                    # PE: transpose P to [k, q] chunks; ACT: evacuate
                    ptp = pt_ps.tile([128, NKT, 128], BF16, tag="ptp")
                    for c in range(NKT):
                        nc.tensor.transpose(ptp[:, c, :], p_sb[:, 128 * c:128 * c + 128],
                                            identb[:])
                    pT = att.tile([128, NKT, 128], BF16, tag="pT")
                    nc.scalar.copy(pT[:], ptp[:])
                    # PE: AV accumulate over k chunks
                    av = av_ps.tile([128, DK], F32, tag="av")
                    for c in range(NKT):
                        nc.tensor.matmul(av[:], pT[:, c, :], v_sb[:, c, 64 * h:64 * h + 64],
                                         start=(c == 0), stop=(c == NKT - 1))
                    # DVE: out_h = av/den + bv_h
                    recip = att.tile([128, 1], F32, tag="recip")
                    nc.vector.reciprocal(recip[:], den[:])
                    nc.vector.scalar_tensor_tensor(
                        out=o_parts[:, h, :], in0=av[:], scalar=recip[:],
                        in1=bv_b[:, 64 * h:64 * h + 64], op0=ALU.mult, op1=ALU.add)

                for qt in range(NQT):
                    DVE_HEADS = dve_heads(qt)
                    pe_heads = [h for h in range(H) if h not in DVE_HEADS]
                    o_parts = att.tile([128, H, DK], F32, tag="o", name=f"o_{qt}")
                    masks = masks_by_qt.pop(qt)
                    tail_q = []      # (h, p_sb, den)
                    chains = []      # [ops_list, finish, h]
                    for i, h in enumerate(pe_heads):
                        tail_q.append((h,) + emit_front(qt, h, masks))
                        # qt0 only: slip 2 V-projection groups in after each front
                        for _ in range(2):
                            if vgroups:
                                vgroups.pop(0)()
                        if i < len(DVE_HEADS):
                            ops, fin = make_chain_ops(qt, DVE_HEADS[i], masks)
                            chains.append([list(ops), fin, DVE_HEADS[i]])
                            chains[-1][0].pop(0)()  # first STT frees the qk bank
                        else:
                            # drain chain STT ops, alternating between chains
                            nslots = len(pe_heads) - len(DVE_HEADS)
                            ndrain = -(-(8 * len(DVE_HEADS)) // nslots)
                            for _ in range(ndrain):
                                for ch in chains:
                                    if ch[0]:
                                        ch[0].pop(0)()
                                        break
                            for ch in [c for c in chains if not c[0] and c[1]]:
                                tail_q.append((ch[2],) + ch[1]())
                                ch[1] = None
                            for _ in range(2 if i >= 6 else 1):
                                if tail_q:
                                    th, tp, td = tail_q.pop(0)
                                    emit_tail(qt, th, tp, td, o_parts)

                    # flush remaining chain ops / finishes / tails
                    for ch in chains:
                        while ch[0]:
                            ch[0].pop(0)()
                        if ch[1]:
                            tail_q.append((ch[2],) + ch[1]())
                            ch[1] = None
                    for th, tp, td in tail_q:
                        emit_tail(qt, th, tp, td, o_parts)
                    if qt + 2 < NQT:
                        masks_by_qt[qt + 2] = emit_prep(qt + 2)
                    nc.sync.dma_start(out=out[128 * qt:128 * qt + 128, :],
                                      in_=o_parts[:].rearrange("p h d -> p (h d)"))
    nc.compile()
    return nc


_NC_CACHE = {}


def _get_nc():
    if "nc" not in _NC_CACHE:
        _NC_CACHE["nc"] = build_nc()
    return _NC_CACHE["nc"]


def _make_in_maps(inputs):
    q = np.asarray(inputs["query"], dtype=np.float32)
    k = np.asarray(inputs["key"], dtype=np.float32)
    v = np.asarray(inputs["value"], dtype=np.float32)
    tp = np.ascontiguousarray(np.asarray(inputs["tile_positions"], dtype=np.float32))
    f16 = lambda a: np.ascontiguousarray(a.astype(np.float16))
    wqT = f16(np.asarray(inputs["Wq"], dtype=np.float32).T)
    wkT = f16(np.asarray(inputs["Wk"], dtype=np.float32).T)
    wvT = f16(np.asarray(inputs["Wv"], dtype=np.float32).T)
    embT = f16(np.asarray(inputs["emb_k"], dtype=np.float32).T)
    bqa = np.ascontiguousarray(np.asarray(inputs["bq"], dtype=np.float32))
    bka = np.ascontiguousarray(np.asarray(inputs["bk"], dtype=np.float32))
    bva = np.ascontiguousarray(np.asarray(inputs["bv"], dtype=np.float32))
    xkT = [f16(k[b].T) for b in range(B)]
    xvT = [f16(v[b].T) for b in range(B)]
    in_maps = []
    for c in range(NCORES):
        b, qh = c // 2, c % 2
        in_maps.append({
            "xqT": f16(q[b, qh * SQ:(qh + 1) * SQ].T),
            "xkT": xkT[b], "xvT": xvT[b],
            "pos": tp[b],
            "posq": np.ascontiguousarray(tp[b, qh * SQ:(qh + 1) * SQ]),
            "wqT": wqT, "wkT": wkT, "wvT": wvT,
            "bq": bqa, "bk": bka, "bv": bva,
            "embT": embT,
        })
    return in_maps


def kernel(query, key, value, tile_positions, Wq, bq, Wk, bk, Wv, bv, emb_k):
    inputs = {"query": query, "key": key, "value": value,
              "tile_positions": tile_positions,
              "Wq": Wq, "bq": bq, "Wk": Wk, "bk": bk, "Wv": Wv, "bv": bv,
              "emb_k": emb_k}
    nc = _get_nc()
    in_maps = _make_in_maps(inputs)
    res = run_bass_kernel_spmd(nc, in_maps, core_ids=list(range(NCORES)))
    out = np.empty((B, S, D), np.float32)
    for c in range(NCORES):
        b, qh = c // 2, c % 2
        out[b, qh * SQ:(qh + 1) * SQ] = res.results[c]["out"]
    return out
